# revision 6
# baseline (speedup 1.0000x reference)
"""DecoderRNN Trainium2 kernel (data-parallel over 8 NeuronCores).

Computation (see reference): 2 branches x 10 steps x 2 LSTM cells, strictly
sequential (h, c, prev carry through all 40 cell evals), batch 4096, hidden
1024, input 2048+128+128.

Structure per core (batch shard BL=512), all in "transposed" layout
(partition dim = feature, free dim = batch) so the recurrence needs no
transposes:

  - Zpre[b] = W_z @ z_b.T  (4096, BL) precomputed once per branch on device
    (z/bid/bias parts of the input projection are step-invariant; bid+biases
    folded into a per-partition bias vector applied by ScalarE).
  - action_emb contribution -> 16-row table T_act = action_emb @ W_act.T,
    applied per step as a K=16 one-hot matmul accumulated into the same PSUM
    group as the h @ W_hh.T tiles.
  - gates.T accumulate in PSUM (32 M-tiles x 9 K-tiles, N=512, bf16);
    VectorE adds streamed Zpre, ScalarE applies sigmoid/tanh with bias,
    VectorE forms c and h (h written directly as bf16 for the next matmul).
  - tiny heads (4/11 logits) matmul'd per step; log-softmax DEFERRED: logits
    stored packed (4 steps per 32-partition group), one Exp/Ln table switch
    at the end, partition-group reduction via fp32 matmul with a 0/1 matrix.

Host side does only index/layout prep: weight transposes + bf16 casts,
one-hot encodings of the (input) action sequences, and the final
(2,10,BL) -> (BL,2,10) transpose of the gathered outputs.
"""

import numpy as np
import ml_dtypes

import concourse.bass as bass
import concourse.bacc as bacc
import concourse.mybir as mybir
import concourse.tile as tile
from concourse.bass_utils import run_bass_kernel_spmd

f32 = mybir.dt.float32
bf16 = mybir.dt.bfloat16
AF = mybir.ActivationFunctionType

B, ENC, DEC, EMB, S = 4096, 2048, 1024, 128, 10
NT, NM = 4, 11
VOCAB = NT + NM + 1  # 16, start token 15
NCORES = 8
BL = B // NCORES     # 512 batch rows per core
KZ = ENC // 128      # 16 K-tiles for z projection
KH = DEC // 128      # 8 K-tiles for h projection
MT = 4 * DEC // 128  # 32 M-tiles over the gate dim
NCALL = 2 * S * 2    # 40 sequential LSTM cell evals
NBLK = 2 * S         # 20 (branch, step) head blocks
NQ = (NBLK + 3) // 4  # 5 column groups of 4 blocks in packed logit layout

_CACHE = {}


def _build_program(trace_friendly=False):
    nc = bacc.Bacc("TRN2")

    zT_d = nc.dram_tensor("zT", (128, 2, KZ, BL), bf16, kind="ExternalInput")
    wzT_d = nc.dram_tensor("wzT", (128, KZ, 4 * DEC), bf16, kind="ExternalInput")
    whhT_d = nc.dram_tensor("whhT", (128, KH, 4 * DEC), bf16, kind="ExternalInput")
    tact_d = nc.dram_tensor("tact", (VOCAB, 4 * DEC), bf16, kind="ExternalInput")
    ball_d = nc.dram_tensor("ball", (128, MT), f32, kind="ExternalInput")
    oneh_d = nc.dram_tensor("oneh", (NCALL, VOCAB, BL), bf16, kind="ExternalInput")
    oselt_d = nc.dram_tensor("oselt", (128, NQ * BL), f32, kind="ExternalInput")
    oselm_d = nc.dram_tensor("oselm", (128, NQ * BL), f32, kind="ExternalInput")
    wtT_d = nc.dram_tensor("wtT", (128, KH, NT), bf16, kind="ExternalInput")
    wmT_d = nc.dram_tensor("wmT", (128, KH, NM), bf16, kind="ExternalInput")
    bt_d = nc.dram_tensor("bt", (NT, 1), f32, kind="ExternalInput")
    bm_d = nc.dram_tensor("bm", (NM, 1), f32, kind="ExternalInput")
    redt_d = nc.dram_tensor("redt", (128, 4), f32, kind="ExternalInput")
    redm_d = nc.dram_tensor("redm", (128, 4), f32, kind="ExternalInput")

    tout_d = nc.dram_tensor("tout", (2, S, BL), f32, kind="ExternalOutput")
    mout_d = nc.dram_tensor("mout", (2, S, BL), f32, kind="ExternalOutput")

    with tile.TileContext(nc) as tc:
        with tc.tile_pool(name="res", bufs=1) as res, \
             tc.tile_pool(name="hbuf", bufs=2) as hbuf, \
             tc.tile_pool(name="dram", bufs=1, space="DRAM") as dram:

            whh_s = res.tile([128, KH, 4 * DEC], bf16)
            nc.sync.dma_start(whh_s[:], whhT_d[:])
            tact_s = res.tile([VOCAB, 4 * DEC], bf16)
            nc.sync.dma_start(tact_s[:], tact_d[:])
            ball_s = res.tile([128, MT], f32)
            nc.sync.dma_start(ball_s[:], ball_d[:])
            wtT_s = res.tile([128, KH, NT], bf16)
            nc.sync.dma_start(wtT_s[:], wtT_d[:])
            wmT_s = res.tile([128, KH, NM], bf16)
            nc.sync.dma_start(wmT_s[:], wmT_d[:])
            bt_s = res.tile([NT, 1], f32)
            nc.sync.dma_start(bt_s[:], bt_d[:])
            bm_s = res.tile([NM, 1], f32)
            nc.sync.dma_start(bm_s[:], bm_d[:])

            # packed logit buffers: block (b*10+s) = 4q+r lives at
            # partitions 32r.. (4 rows for t, 11 for m), free cols q*BL..
            lt_s = res.tile([128, NQ * BL], f32)
            lm_s = res.tile([128, NQ * BL], f32)
            nc.vector.memset(lt_s[:], 0.0)
            nc.vector.memset(lm_s[:], 0.0)

            # persistent cell state (fp32) and ping-pong bf16 h
            c_s = res.tile([128, KH, BL], f32)
            nc.vector.memset(c_s[:], 0.0)
            h_prev = hbuf.tile([128, KH, BL], bf16, tag="h")
            nc.vector.memset(h_prev[:], 0.0)

            # chunk-major layout: [br, p, j, g, n] holds gate m-tile (g*KH+j)
            zpre_t = dram.tile([2, 128, KH, 4, BL], bf16)

            # ---------------- phase 1: Zpre = Wz @ z.T (both branches) ----
            with tc.tile_pool(name="p1", bufs=1) as p1, \
                 tc.tile_pool(name="p1wz", bufs=6) as p1wz, \
                 tc.tile_pool(name="p1dr", bufs=8) as p1dr, \
                 tc.tile_pool(name="p1ps", bufs=8, space="PSUM") as p1ps:
                zt_s = p1.tile([128, 2, KZ, BL], bf16)
                nc.sync.dma_start(zt_s[:], zT_d[:])
                for mg in range(4):          # m-groups of 8 M-tiles
                    for br in range(2):
                        psums = []
                        for m8 in range(8):
                            psums.append(p1ps.tile([128, BL], f32, tag="zp_ps", name="zp_ps"))
                        for k in range(KZ):
                            wz_k = p1wz.tile([128, 8 * 128], bf16, tag="wz")
                            nc.sync.dma_start(
                                wz_k[:], wzT_d[:, k, mg * 1024:(mg + 1) * 1024])
                            for m8 in range(8):
                                nc.tensor.matmul(
                                    psums[m8][:],
                                    wz_k[:, m8 * 128:(m8 + 1) * 128],
                                    zt_s[:, br, k, :],
                                    start=(k == 0), stop=(k == KZ - 1))
                        for m8 in range(8):
                            m = mg * 8 + m8
                            zdr = p1dr.tile([128, BL], bf16, tag="zdr")
                            nc.scalar.activation(zdr[:], psums[m8][:], AF.Identity)
                            nc.sync.dma_start(
                                zpre_t[br, :, m % KH, m // KH, :], zdr[:])

            # ---------------- phase 2: 40 sequential LSTM cells -----------
            with tc.tile_pool(name="zpst", bufs=10) as zpst, \
                 tc.tile_pool(name="ohp", bufs=4) as ohp, \
                 tc.tile_pool(name="tmpp", bufs=4) as tmpp, \
                 tc.tile_pool(name="actp", bufs=4) as actp, \
                 tc.tile_pool(name="cmix", bufs=3) as cmix, \
                 tc.tile_pool(name="tcp", bufs=2) as tcp, \
                 tc.tile_pool(name="ps", bufs=8, space="PSUM") as ps:
                for li in range(NCALL):
                    br = li // (2 * S)
                    s = (li % (2 * S)) // 2
                    is_m = li % 2  # 0: transform head, 1: magnitude head

                    oh = ohp.tile([VOCAB, BL], bf16, tag="oh")
                    nc.sync.dma_start(oh[:], oneh_d[li])

                    h_new = hbuf.tile([128, KH, BL], bf16, tag="h")
                    for j in range(KH):  # hidden chunk
                        zp = zpst.tile([128, 4, BL], bf16, tag="zp")
                        nc.sync.dma_start(zp[:], zpre_t[br, :, j, :, :])
                        acts = []
                        for g in range(4):  # i, f, g, o
                            m = g * KH + j
                            pt = ps.tile([128, BL], f32, tag="ps")
                            nc.tensor.matmul(
                                pt[:], tact_s[:, m * 128:(m + 1) * 128], oh[:],
                                start=True, stop=False)
                            for k in range(KH):
                                nc.tensor.matmul(
                                    pt[:],
                                    whh_s[:, k, m * 128:(m + 1) * 128],
                                    h_prev[:, k, :],
                                    start=False, stop=(k == KH - 1))
                            tmp = tmpp.tile([128, BL], f32, tag="tmp")
                            nc.vector.tensor_add(
                                tmp[:], pt[:], zp[:, g, :])
                            av = actp.tile([128, BL], f32, tag="act")
                            fn = AF.Tanh if g == 2 else AF.Sigmoid
                            nc.scalar.activation(
                                av[:], tmp[:], fn, bias=ball_s[:, m:m + 1])
                            acts.append(av)
                        a_i, a_f, a_g, a_o = acts
                        cf = cmix.tile([128, BL], f32, tag="cf")
                        nc.vector.tensor_mul(cf[:], a_f[:], c_s[:, j, :])
                        ci = cmix.tile([128, BL], f32, tag="ci")
                        nc.vector.tensor_mul(ci[:], a_i[:], a_g[:])
                        nc.vector.tensor_add(c_s[:, j, :], cf[:], ci[:])
                        tch = tcp.tile([128, BL], f32, tag="tc")
                        nc.scalar.activation(tch[:], c_s[:, j, :], AF.Tanh)
                        nc.vector.tensor_mul(h_new[:, j, :], a_o[:], tch[:])

                    # head on h_new
                    nh = NM if is_m else NT
                    wh = wmT_s if is_m else wtT_s
                    bh = bm_s if is_m else bt_s
                    ldst = lm_s if is_m else lt_s
                    hd = ps.tile([128, BL], f32, tag="ps")
                    for k in range(KH):
                        nc.tensor.matmul(
                            hd[0:nh, :], wh[:, k, :], h_new[:, k, :],
                            start=(k == 0), stop=(k == KH - 1))
                    blk = br * S + s
                    q, r = divmod(blk, 4)
                    nc.scalar.activation(
                        ldst[32 * r:32 * r + nh, q * BL:(q + 1) * BL],
                        hd[0:nh, :], AF.Identity, bias=bh[:])
                    h_prev = h_new

            # ---------------- phase 3: deferred log-softmax ---------------
            with tc.tile_pool(name="epi", bufs=1) as epi, \
                 tc.tile_pool(name="expp", bufs=4) as expp, \
                 tc.tile_pool(name="lpp", bufs=6) as lpp, \
                 tc.tile_pool(name="eps", bufs=4, space="PSUM") as eps:
                oselt_s = epi.tile([128, NQ * BL], f32)
                nc.sync.dma_start(oselt_s[:], oselt_d[:])
                oselm_s = epi.tile([128, NQ * BL], f32)
                nc.sync.dma_start(oselm_s[:], oselm_d[:])
                redt_s = epi.tile([128, 4], f32)
                nc.sync.dma_start(redt_s[:], redt_d[:])
                redm_s = epi.tile([128, 4], f32)
                nc.sync.dma_start(redm_s[:], redm_d[:])

                for is_m in range(2):
                    lsrc = lm_s if is_m else lt_s
                    osel = oselm_s if is_m else oselt_s
                    red = redm_s if is_m else redt_s
                    out_d = mout_d if is_m else tout_d
                    # dst view: block (b*10+s) = 4q+r -> [r, q, n]
                    dst = out_d[:].rearrange("b s n -> (b s) n").rearrange(
                        "(q r) n -> r q n", r=4)
                    for q in range(NQ):
                        col = slice(q * BL, (q + 1) * BL)
                        ex = expp.tile([128, BL], f32, tag="ex")
                        nc.scalar.activation(ex[:], lsrc[:, col], AF.Exp)
                        pr = expp.tile([128, BL], f32, tag="pr")
                        nc.vector.tensor_mul(pr[:], lsrc[:, col], osel[:, col])
                        se = eps.tile([4, BL], f32, tag="eps")
                        nc.tensor.matmul(se[:], red[:], ex[:],
                                         start=True, stop=True)
                        lnz = lpp.tile([4, BL], f32, tag="lnz")
                        nc.scalar.activation(lnz[:], se[:], AF.Ln)
                        ch = eps.tile([4, BL], f32, tag="eps")
                        nc.tensor.matmul(ch[:], red[:], pr[:],
                                         start=True, stop=True)
                        lp = lpp.tile([4, BL], f32, tag="lp")
                        nc.vector.tensor_sub(lp[:], ch[:], lnz[:])
                        nc.sync.dma_start(dst[:, q, :], lp[:])

    nc.finalize()
    return nc


def _prep_core_inputs(z1, z2, old_transform, old_magnitude, shared, core):
    sl = slice(core * BL, (core + 1) * BL)
    ot = old_transform[sl]   # (BL, 2, S) int32
    om = old_magnitude[sl]

    def ztile(z):
        # (BL, ENC) -> (128, KZ, BL) bf16, [p, k, n] = z[n, k*128+p]
        zt = np.ascontiguousarray(z[sl].T).reshape(KZ, 128, BL)
        return zt.transpose(1, 0, 2)

    zT = np.stack([ztile(z1), ztile(z2)], axis=1)  # (128, 2, KZ, BL)

    # per-call action index -> one-hot (VOCAB, BL)
    oneh = np.zeros((NCALL, VOCAB, BL), np.float32)
    cols = np.arange(BL)
    prev = np.full(BL, VOCAB - 1, np.int64)  # start token
    li = 0
    for br in range(2):
        for s in range(S):
            oneh[li, prev, cols] = 1.0          # transform cell input
            li += 1
            a = ot[:, br, s].astype(np.int64)
            oneh[li, a, cols] = 1.0             # magnitude cell input
            li += 1
            prev = om[:, br, s].astype(np.int64)

    # packed head-selection one-hots
    oselt = np.zeros((128, NQ * BL), np.float32)
    oselm = np.zeros((128, NQ * BL), np.float32)
    for br in range(2):
        for s in range(S):
            q, r = divmod(br * S + s, 4)
            oselt[32 * r + ot[:, br, s].astype(np.int64), q * BL + cols] = 1.0
            oselm[32 * r + om[:, br, s].astype(np.int64), q * BL + cols] = 1.0

    m = {
        "zT": zT.astype(ml_dtypes.bfloat16),
        "oneh": oneh.astype(ml_dtypes.bfloat16),
        "oselt": oselt,
        "oselm": oselm,
    }
    m.update(shared)
    return m


def _prep_shared(action_emb, branch_emb, W_ih, W_hh, b_ih, b_hh, Wt, bt, Wm, bm):
    Wz = W_ih[:, :ENC]
    Wbid = W_ih[:, ENC:ENC + EMB]
    Wact = W_ih[:, ENC + EMB:]
    b_all = (b_ih + b_hh + Wbid @ branch_emb[0]).astype(np.float32)

    def kt(WT, kdim, mdim):
        # (mdim, kdim) weight -> lhsT tiles (128, kdim/128, mdim)
        return np.ascontiguousarray(
            WT.T.reshape(kdim // 128, 128, mdim).transpose(1, 0, 2))

    redt = np.zeros((128, 4), np.float32)
    redm = np.zeros((128, 4), np.float32)
    for r in range(4):
        redt[32 * r:32 * r + NT, r] = 1.0
        redm[32 * r:32 * r + NM, r] = 1.0

    return {
        "wzT": kt(Wz, ENC, 4 * DEC).astype(ml_dtypes.bfloat16),
        "whhT": kt(W_hh, DEC, 4 * DEC).astype(ml_dtypes.bfloat16),
        "tact": (action_emb @ Wact.T).astype(ml_dtypes.bfloat16),
        "ball": np.ascontiguousarray(b_all.reshape(MT, 128).T),
        "wtT": kt(Wt, DEC, NT).astype(ml_dtypes.bfloat16),
        "wmT": kt(Wm, DEC, NM).astype(ml_dtypes.bfloat16),
        "bt": bt.reshape(NT, 1).astype(np.float32),
        "bm": bm.reshape(NM, 1).astype(np.float32),
        "redt": redt,
        "redm": redm,
    }


def kernel(z1, z2, action_emb, branch_emb, W_ih, W_hh, b_ih, b_hh,
           Wt, bt, Wm, bm, old_transform, old_magnitude,
           _trace=False, _tmpdir=None):
    if "nc" not in _CACHE:
        _CACHE["nc"] = _build_program()
    nc = _CACHE["nc"]

    z1 = np.asarray(z1, np.float32)
    z2 = np.asarray(z2, np.float32)
    shared = _prep_shared(np.asarray(action_emb, np.float32),
                          np.asarray(branch_emb, np.float32),
                          np.asarray(W_ih, np.float32),
                          np.asarray(W_hh, np.float32),
                          np.asarray(b_ih, np.float32),
                          np.asarray(b_hh, np.float32),
                          np.asarray(Wt, np.float32),
                          np.asarray(bt, np.float32),
                          np.asarray(Wm, np.float32),
                          np.asarray(bm, np.float32))
    old_transform = np.asarray(old_transform)
    old_magnitude = np.asarray(old_magnitude)
    in_maps = [
        _prep_core_inputs(z1, z2, old_transform, old_magnitude, shared, c)
        for c in range(NCORES)
    ]

    kw = {}
    if _trace:
        kw = dict(trace=True, tmpdir=_tmpdir)
    out = run_bass_kernel_spmd(nc, in_maps, core_ids=list(range(NCORES)), **kw)

    t_lp = np.concatenate(
        [r["tout"].transpose(2, 0, 1) for r in out.results], axis=0)
    m_lp = np.concatenate(
        [r["mout"].transpose(2, 0, 1) for r in out.results], axis=0)
    res = (old_transform, t_lp, old_magnitude, m_lp)
    if _trace:
        return res, out
    return res


# revision 12
# speedup vs baseline: 1.4941x; 1.4941x over previous
"""DecoderRNN Trainium2 kernel (data-parallel over 8 NeuronCores).

Computation (see reference): 2 branches x 10 steps x 2 LSTM cells, strictly
sequential (h, c, prev carry through all 40 cell evals), batch 4096, hidden
1024, input 2048+128+128.

Structure per core (batch shard BL=512), all in "transposed" layout
(partition dim = feature, free dim = batch) so the recurrence needs no
transposes:

  - Zpre[b] = W_z @ z_b.T  (4096, BL) precomputed once per branch on device
    (z/bid/bias parts of the input projection are step-invariant; bid+biases
    folded into a per-partition bias vector applied by ScalarE).
  - action_emb contribution -> 16-row table T_act = action_emb @ W_act.T,
    applied per step as a K=16 one-hot matmul accumulated into the same PSUM
    group as the h @ W_hh.T tiles.
  - gates.T accumulate in PSUM (32 M-tiles x 9 K-tiles, N=512, bf16);
    VectorE adds streamed Zpre, ScalarE applies sigmoid/tanh with bias,
    VectorE forms c and h (h written directly as bf16 for the next matmul).
  - tiny heads (4/11 logits) matmul'd per step; log-softmax DEFERRED: logits
    stored packed (4 steps per 32-partition group), one Exp/Ln table switch
    at the end, partition-group reduction via fp32 matmul with a 0/1 matrix.

Host side does only index/layout prep: weight transposes + bf16 casts,
one-hot encodings of the (input) action sequences, and the final
(2,10,BL) -> (BL,2,10) transpose of the gathered outputs.
"""

import numpy as np
import ml_dtypes

import concourse.bass as bass
import concourse.bacc as bacc
import concourse.mybir as mybir
import concourse.tile as tile
from concourse.bass_utils import run_bass_kernel_spmd

f32 = mybir.dt.float32
bf16 = mybir.dt.bfloat16
AF = mybir.ActivationFunctionType

B, ENC, DEC, EMB, S = 4096, 2048, 1024, 128, 10
NT, NM = 4, 11
VOCAB = NT + NM + 1  # 16, start token 15
NCORES = 8
BL = B // NCORES     # 512 batch rows per core
KZ = ENC // 128      # 16 K-tiles for z projection
KH = DEC // 128      # 8 K-tiles for h projection
MT = 4 * DEC // 128  # 32 M-tiles over the gate dim
NCALL = 2 * S * 2    # 40 sequential LSTM cell evals
NBLK = 2 * S         # 20 (branch, step) head blocks
NQ = (NBLK + 3) // 4  # 5 column groups of 4 blocks in packed logit layout

_CACHE = {}


def _build_program(trace_friendly=False, ncall=NCALL, do_p1=True, do_epi=True,
                   rowtile=True):
    nc = bacc.Bacc("TRN2")

    zT_d = nc.dram_tensor("zT", (128, 2, KZ, BL), bf16, kind="ExternalInput")
    wzT_d = nc.dram_tensor("wzT", (128, KZ, 4 * DEC), bf16, kind="ExternalInput")
    whhT_d = nc.dram_tensor("whhT", (128, KH, 4 * DEC), bf16, kind="ExternalInput")
    tact_d = nc.dram_tensor("tact", (128, 4 * DEC), bf16, kind="ExternalInput")
    ball_d = nc.dram_tensor("ball", (128, MT), f32, kind="ExternalInput")
    oneh_d = nc.dram_tensor("oneh", (NCALL, 128, BL), bf16, kind="ExternalInput")
    oselt_d = nc.dram_tensor("oselt", (128, NQ * BL), f32, kind="ExternalInput")
    oselm_d = nc.dram_tensor("oselm", (128, NQ * BL), f32, kind="ExternalInput")
    wtT_d = nc.dram_tensor("wtT", (128, KH, NT), bf16, kind="ExternalInput")
    wmT_d = nc.dram_tensor("wmT", (128, KH, NM), bf16, kind="ExternalInput")
    bt_d = nc.dram_tensor("bt", (NT, 1), f32, kind="ExternalInput")
    bm_d = nc.dram_tensor("bm", (NM, 1), f32, kind="ExternalInput")
    redt_d = nc.dram_tensor("redt", (128, 4), f32, kind="ExternalInput")
    redm_d = nc.dram_tensor("redm", (128, 4), f32, kind="ExternalInput")

    tout_d = nc.dram_tensor("tout", (2, S, BL), f32, kind="ExternalOutput")
    mout_d = nc.dram_tensor("mout", (2, S, BL), f32, kind="ExternalOutput")

    with tile.TileContext(nc) as tc:
        with tc.tile_pool(name="res", bufs=1) as res, \
             tc.tile_pool(name="hbuf", bufs=2) as hbuf, \
             tc.tile_pool(name="dram", bufs=1, space="DRAM") as dram:

            whh_s = res.tile([128, KH, 4 * DEC], bf16)
            nc.sync.dma_start(whh_s[:], whhT_d[:])
            tact_s = res.tile([128, 4 * DEC], bf16)
            nc.sync.dma_start(tact_s[:], tact_d[:])
            ball_s = res.tile([128, MT], f32)
            nc.sync.dma_start(ball_s[:], ball_d[:])
            wtT_s = res.tile([128, KH, NT], bf16)
            nc.sync.dma_start(wtT_s[:], wtT_d[:])
            wmT_s = res.tile([128, KH, NM], bf16)
            nc.sync.dma_start(wmT_s[:], wmT_d[:])
            bt_s = res.tile([NT, 1], f32)
            nc.sync.dma_start(bt_s[:], bt_d[:])
            bm_s = res.tile([NM, 1], f32)
            nc.sync.dma_start(bm_s[:], bm_d[:])

            # packed logit buffers: block (b*10+s) = 4q+r lives at
            # partitions 32r.. (4 rows for t, 11 for m), free cols q*BL..
            lt_s = res.tile([128, NQ * BL], f32)
            lm_s = res.tile([128, NQ * BL], f32)
            nc.vector.memset(lt_s[:], 0.0)
            nc.vector.memset(lm_s[:], 0.0)

            # persistent cell state (fp32) and ping-pong bf16 h
            c_s = res.tile([128, KH, BL], f32)
            nc.vector.memset(c_s[:], 0.0)
            h_prev = hbuf.tile([128, KH, BL], bf16, tag="h")
            nc.vector.memset(h_prev[:], 0.0)

            # chunk-major layout: [br, p, j, g, n] holds gate m-tile (g*KH+j)
            zpre_t = dram.tile([2, 128, KH, 4, BL], bf16)

            # ---------------- phase 1: Zpre = Wz @ z.T (both branches) ----
            with tc.tile_pool(name="p1", bufs=1) as p1, \
                 tc.tile_pool(name="p1wz", bufs=6) as p1wz, \
                 tc.tile_pool(name="p1dr", bufs=8) as p1dr, \
                 tc.tile_pool(name="p1ps", bufs=8, space="PSUM") as p1ps:
                zt_s = p1.tile([128, 2, KZ, BL], bf16)
                nc.sync.dma_start(zt_s[:], zT_d[:])
                for mg in range(4 if do_p1 else 0):          # m-groups of 8 M-tiles
                    for br in range(2):
                        psums = []
                        for m8 in range(8):
                            psums.append(p1ps.tile([128, BL], f32, tag="zp_ps", name="zp_ps"))
                        for k in range(KZ):
                            wz_k = p1wz.tile([128, 8 * 128], bf16, tag="wz")
                            nc.sync.dma_start(
                                wz_k[:], wzT_d[:, k, mg * 1024:(mg + 1) * 1024])
                            for m8 in range(8):
                                nc.tensor.matmul(
                                    psums[m8][:],
                                    wz_k[:, m8 * 128:(m8 + 1) * 128],
                                    zt_s[:, br, k, :],
                                    start=(k == 0), stop=(k == KZ - 1))
                        for m8 in range(8):
                            m = mg * 8 + m8
                            zdr = p1dr.tile([128, BL], bf16, tag="zdr")
                            nc.scalar.activation(zdr[:], psums[m8][:], AF.Identity)
                            nc.sync.dma_start(
                                zpre_t[br, :, m % KH, m // KH, :], zdr[:])

            # ---------------- phase 2: 40 sequential LSTM cells -----------
            with tc.tile_pool(name="zpst", bufs=10) as zpst, \
                 tc.tile_pool(name="ohp", bufs=4) as ohp, \
                 tc.tile_pool(name="tmpp", bufs=4) as tmpp, \
                 tc.tile_pool(name="actp", bufs=4) as actp, \
                 tc.tile_pool(name="cmix", bufs=3) as cmix, \
                 tc.tile_pool(name="tcp", bufs=2) as tcp, \
                 tc.tile_pool(name="ps", bufs=8, space="PSUM") as ps:
                for li in range(ncall):
                    br = (li // (2 * S)) % 2
                    s = (li % (2 * S)) // 2
                    is_m = li % 2  # 0: transform head, 1: magnitude head

                    oh = ohp.tile([128, BL], bf16, tag="oh")
                    nc.sync.dma_start(oh[:], oneh_d[li % NCALL])

                    h_new = hbuf.tile([128, KH, BL], bf16, tag="h")
                    for j in range(KH):  # hidden chunk
                        zp = zpst.tile([128, 4, BL], bf16, tag="zp")
                        nc.sync.dma_start(zp[:], zpre_t[br, :, j, :, :])
                        acts = []
                        if rowtile:
                            # 4 concurrent K=16 one-hot matmuls, one per gate
                            # bank, on disjoint 32-row strips of the PE array
                            pts = []
                            for g in range(4):
                                m = g * KH + j
                                pt = ps.tile([128, BL], f32, tag="ps",
                                             name="pt")
                                nc.tensor.matmul(
                                    pt[:],
                                    tact_s[32 * g:32 * g + VOCAB,
                                           m * 128:(m + 1) * 128],
                                    oh[32 * g:32 * g + VOCAB, :],
                                    start=True, stop=False,
                                    tile_position=(32 * g, 0))
                                pts.append(pt)
                            for k in range(KH):
                                for g in range(4):
                                    m = g * KH + j
                                    nc.tensor.matmul(
                                        pts[g][:],
                                        whh_s[:, k, m * 128:(m + 1) * 128],
                                        h_prev[:, k, :],
                                        start=False, stop=(k == KH - 1))
                            for g in range(4):
                                m = g * KH + j
                                tmp = tmpp.tile([128, BL], f32, tag="tmp")
                                nc.vector.tensor_add(
                                    tmp[:], pts[g][:], zp[:, g, :])
                                av = actp.tile([128, BL], f32, tag="act")
                                fn = AF.Tanh if g == 2 else AF.Sigmoid
                                nc.scalar.activation(
                                    av[:], tmp[:], fn, bias=ball_s[:, m:m + 1])
                                acts.append(av)
                        else:
                            for g in range(4):  # i, f, g, o
                                m = g * KH + j
                                pt = ps.tile([128, BL], f32, tag="ps",
                                             name="pt")
                                nc.tensor.matmul(
                                    pt[:], tact_s[0:VOCAB, m * 128:(m + 1) * 128],
                                    oh[0:VOCAB, :],
                                    start=True, stop=False)
                                for k in range(KH):
                                    nc.tensor.matmul(
                                        pt[:],
                                        whh_s[:, k, m * 128:(m + 1) * 128],
                                        h_prev[:, k, :],
                                        start=False, stop=(k == KH - 1))
                                tmp = tmpp.tile([128, BL], f32, tag="tmp")
                                nc.vector.tensor_add(
                                    tmp[:], pt[:], zp[:, g, :])
                                av = actp.tile([128, BL], f32, tag="act")
                                fn = AF.Tanh if g == 2 else AF.Sigmoid
                                nc.scalar.activation(
                                    av[:], tmp[:], fn, bias=ball_s[:, m:m + 1])
                                acts.append(av)
                        a_i, a_f, a_g, a_o = acts
                        cf = cmix.tile([128, BL], f32, tag="cf")
                        nc.vector.tensor_mul(cf[:], a_f[:], c_s[:, j, :])
                        ci = cmix.tile([128, BL], f32, tag="ci")
                        nc.vector.tensor_mul(ci[:], a_i[:], a_g[:])
                        nc.vector.tensor_add(c_s[:, j, :], cf[:], ci[:])
                        tch = tcp.tile([128, BL], f32, tag="tc")
                        nc.scalar.activation(tch[:], c_s[:, j, :], AF.Tanh)
                        nc.vector.tensor_mul(h_new[:, j, :], a_o[:], tch[:])

                    # head on h_new
                    nh = NM if is_m else NT
                    wh = wmT_s if is_m else wtT_s
                    bh = bm_s if is_m else bt_s
                    ldst = lm_s if is_m else lt_s
                    hd = ps.tile([128, BL], f32, tag="ps")
                    for k in range(KH):
                        nc.tensor.matmul(
                            hd[0:nh, :], wh[:, k, :], h_new[:, k, :],
                            start=(k == 0), stop=(k == KH - 1))
                    blk = br * S + s
                    q, r = divmod(blk, 4)
                    nc.scalar.activation(
                        ldst[32 * r:32 * r + nh, q * BL:(q + 1) * BL],
                        hd[0:nh, :], AF.Identity, bias=bh[:])
                    h_prev = h_new

            # ---------------- phase 3: deferred log-softmax ---------------
            with tc.tile_pool(name="epi", bufs=1) as epi, \
                 tc.tile_pool(name="expp", bufs=4) as expp, \
                 tc.tile_pool(name="lpp", bufs=6) as lpp, \
                 tc.tile_pool(name="eps", bufs=4, space="PSUM") as eps:
                oselt_s = epi.tile([128, NQ * BL], f32)
                nc.sync.dma_start(oselt_s[:], oselt_d[:])
                oselm_s = epi.tile([128, NQ * BL], f32)
                nc.sync.dma_start(oselm_s[:], oselm_d[:])
                redt_s = epi.tile([128, 4], f32)
                nc.sync.dma_start(redt_s[:], redt_d[:])
                redm_s = epi.tile([128, 4], f32)
                nc.sync.dma_start(redm_s[:], redm_d[:])

                for is_m in range(2 if do_epi else 0):
                    lsrc = lm_s if is_m else lt_s
                    osel = oselm_s if is_m else oselt_s
                    red = redm_s if is_m else redt_s
                    out_d = mout_d if is_m else tout_d
                    # dst view: block (b*10+s) = 4q+r -> [r, q, n]
                    dst = out_d[:].rearrange("b s n -> (b s) n").rearrange(
                        "(q r) n -> r q n", r=4)
                    for q in range(NQ):
                        col = slice(q * BL, (q + 1) * BL)
                        ex = expp.tile([128, BL], f32, tag="ex")
                        nc.scalar.activation(ex[:], lsrc[:, col], AF.Exp)
                        pr = expp.tile([128, BL], f32, tag="pr")
                        nc.vector.tensor_mul(pr[:], lsrc[:, col], osel[:, col])
                        se = eps.tile([4, BL], f32, tag="eps")
                        nc.tensor.matmul(se[:], red[:], ex[:],
                                         start=True, stop=True)
                        lnz = lpp.tile([4, BL], f32, tag="lnz")
                        nc.scalar.activation(lnz[:], se[:], AF.Ln)
                        ch = eps.tile([4, BL], f32, tag="eps")
                        nc.tensor.matmul(ch[:], red[:], pr[:],
                                         start=True, stop=True)
                        lp = lpp.tile([4, BL], f32, tag="lp")
                        nc.vector.tensor_sub(lp[:], ch[:], lnz[:])
                        nc.sync.dma_start(dst[:, q, :], lp[:])

    nc.finalize()
    return nc


def _prep_core_inputs(z1, z2, old_transform, old_magnitude, shared, core):
    sl = slice(core * BL, (core + 1) * BL)
    ot = old_transform[sl]   # (BL, 2, S) int32
    om = old_magnitude[sl]

    def ztile(z):
        # (BL, ENC) -> (128, KZ, BL) bf16, [p, k, n] = z[n, k*128+p]
        zt = np.ascontiguousarray(z[sl].T).reshape(KZ, 128, BL)
        return zt.transpose(1, 0, 2)

    zT = np.stack([ztile(z1), ztile(z2)], axis=1)  # (128, 2, KZ, BL)

    # per-call action index -> one-hot, replicated on 4x32-partition strips
    oneh = np.zeros((NCALL, 128, BL), np.float32)
    cols = np.arange(BL)
    prev = np.full(BL, VOCAB - 1, np.int64)  # start token
    li = 0
    for br in range(2):
        for s in range(S):
            for g in range(4):
                oneh[li, 32 * g + prev, cols] = 1.0   # transform cell input
            li += 1
            a = ot[:, br, s].astype(np.int64)
            for g in range(4):
                oneh[li, 32 * g + a, cols] = 1.0      # magnitude cell input
            li += 1
            prev = om[:, br, s].astype(np.int64)

    # packed head-selection one-hots
    oselt = np.zeros((128, NQ * BL), np.float32)
    oselm = np.zeros((128, NQ * BL), np.float32)
    for br in range(2):
        for s in range(S):
            q, r = divmod(br * S + s, 4)
            oselt[32 * r + ot[:, br, s].astype(np.int64), q * BL + cols] = 1.0
            oselm[32 * r + om[:, br, s].astype(np.int64), q * BL + cols] = 1.0

    m = {
        "zT": zT.astype(ml_dtypes.bfloat16),
        "oneh": oneh.astype(ml_dtypes.bfloat16),
        "oselt": oselt,
        "oselm": oselm,
    }
    m.update(shared)
    return m


def _prep_shared(action_emb, branch_emb, W_ih, W_hh, b_ih, b_hh, Wt, bt, Wm, bm):
    Wz = W_ih[:, :ENC]
    Wbid = W_ih[:, ENC:ENC + EMB]
    Wact = W_ih[:, ENC + EMB:]
    b_all = (b_ih + b_hh + Wbid @ branch_emb[0]).astype(np.float32)

    def kt(WT, kdim, mdim):
        # (mdim, kdim) weight -> lhsT tiles (128, kdim/128, mdim)
        return np.ascontiguousarray(
            WT.T.reshape(kdim // 128, 128, mdim).transpose(1, 0, 2))

    redt = np.zeros((128, 4), np.float32)
    redm = np.zeros((128, 4), np.float32)
    for r in range(4):
        redt[32 * r:32 * r + NT, r] = 1.0
        redm[32 * r:32 * r + NM, r] = 1.0

    return {
        "wzT": kt(Wz, ENC, 4 * DEC).astype(ml_dtypes.bfloat16),
        "whhT": kt(W_hh, DEC, 4 * DEC).astype(ml_dtypes.bfloat16),
        "tact": np.tile((action_emb @ Wact.T), (8, 1))[:128].astype(
            ml_dtypes.bfloat16),
        "ball": np.ascontiguousarray(b_all.reshape(MT, 128).T),
        "wtT": kt(Wt, DEC, NT).astype(ml_dtypes.bfloat16),
        "wmT": kt(Wm, DEC, NM).astype(ml_dtypes.bfloat16),
        "bt": bt.reshape(NT, 1).astype(np.float32),
        "bm": bm.reshape(NM, 1).astype(np.float32),
        "redt": redt,
        "redm": redm,
    }


def kernel(z1, z2, action_emb, branch_emb, W_ih, W_hh, b_ih, b_hh,
           Wt, bt, Wm, bm, old_transform, old_magnitude,
           _trace=False, _tmpdir=None):
    if "nc" not in _CACHE:
        _CACHE["nc"] = _build_program()
    nc = _CACHE["nc"]

    z1 = np.asarray(z1, np.float32)
    z2 = np.asarray(z2, np.float32)
    shared = _prep_shared(np.asarray(action_emb, np.float32),
                          np.asarray(branch_emb, np.float32),
                          np.asarray(W_ih, np.float32),
                          np.asarray(W_hh, np.float32),
                          np.asarray(b_ih, np.float32),
                          np.asarray(b_hh, np.float32),
                          np.asarray(Wt, np.float32),
                          np.asarray(bt, np.float32),
                          np.asarray(Wm, np.float32),
                          np.asarray(bm, np.float32))
    old_transform = np.asarray(old_transform)
    old_magnitude = np.asarray(old_magnitude)
    in_maps = [
        _prep_core_inputs(z1, z2, old_transform, old_magnitude, shared, c)
        for c in range(NCORES)
    ]

    kw = {}
    if _trace:
        kw = dict(trace=True, tmpdir=_tmpdir)
    out = None
    last_exc = None
    for attempt in range(3):  # transient NRT device errors happen; retry
        try:
            out = run_bass_kernel_spmd(
                nc, in_maps, core_ids=list(range(NCORES)), **kw)
            break
        except Exception as e:
            last_exc = e
            import time as _time
            _time.sleep(5 * (attempt + 1))
    if out is None:
        raise last_exc

    t_lp = np.concatenate(
        [r["tout"].transpose(2, 0, 1) for r in out.results], axis=0)
    m_lp = np.concatenate(
        [r["mout"].transpose(2, 0, 1) for r in out.results], axis=0)
    res = (old_transform, t_lp, old_magnitude, m_lp)
    if _trace:
        return res, out
    return res
